# revision 27
# baseline (speedup 1.0000x reference)
"""MoE (32 experts, top-4, T=64, H=2048, I=1408) — expert-parallel Bass kernel
for 8 trn2 NeuronCores.

Strategy (hardcoded, matches the expert-parallel sharding hint):
  - Each core owns 4 experts; weight shards are shipped pre-transposed and
    pre-tiled into SBUF layout ([128 partitions, chunk, free]) in bf16.
  - x (as x.T, bf16) and router logits are replicated; logits columns are
    permuted per-core so the local experts are columns 0..3.
  - On-device: top-4 + softmax routing weights; routing weights are
    transposed (DVE 32-block transpose) and partition-broadcast so the
    per-token weight can be folded into the up-projection PSUM.
  - Per expert: gate/up computed transposed ([inter, tok]); mixed =
    silu(gate) * up * G[token, e]; the down projection accumulates across
    ALL local experts directly in PSUM, using two column-tiles so tokens
    0..63 land on partitions 0..63 (H 0:1024) and 64..127 (H 1024:2048).
  - Tail: DVE fp32->bf16 copy (2 halves, overlapped with the DRAM DMA),
    ReduceScatter(add) in bf16 over the 8 cores; each core emits a
    [16, 1024] shard (token-strip x H-half); the host reassembles and
    converts to fp32. A tiny dummy ReduceScatter runs early (hidden under
    the weight streaming) to pre-pay the CC engine's per-op setup.
  - Weight DMAs ride three queues (sync=wg, scalar=wu, gpsimd=wd); expert
    0's first h-chunks are fetched in extra-fine groups so the first
    matmul can start ~10us in.
"""

import sys

sys.path.insert(0, "/opt/trn_rl_repo")

import ml_dtypes
import numpy as np

import concourse.bass as bass
import concourse.tile as tile
from concourse import bacc, bass_utils, mybir

T = 64
H = 2048
I = 1408
E = 32
TOPK = 4
NCORES = 8
EPC = E // NCORES  # experts per core
HC = H // 128  # 16 h-chunks
IC = I // 128  # 11 i-chunks

WD_CH = [(0, 4), (4, 4), (8, 3)]  # (start,count) i-chunk groups per down DMA
# gate/up h-chunk DMA groups: finer for expert 0 so the first matmuls can
# start as early as possible, coarse afterwards.
GU_CH0 = [(0, 1), (1, 1), (2, 2), (4, 4), (8, 4), (12, 4)]
GU_CH = [(0, 4), (4, 4), (8, 4), (12, 4)]

f32 = mybir.dt.float32
bf16 = mybir.dt.bfloat16
Alu = mybir.AluOpType
Act = mybir.ActivationFunctionType

_BF16 = np.dtype(ml_dtypes.bfloat16)


def _build_program():
    nc = bacc.Bacc(
        "TRN2",
        target_bir_lowering=False,
        debug=False,
        enable_asserts=False,
        num_devices=NCORES,
    )

    xT_d = nc.dram_tensor("xT", [128, HC, T], bf16, kind="ExternalInput")
    lg_d = nc.dram_tensor("logits", [T, E], f32, kind="ExternalInput")
    wgT_d = nc.dram_tensor("wgT", [EPC, 128, HC, I], bf16, kind="ExternalInput")
    wuT_d = nc.dram_tensor("wuT", [EPC, 128, HC, I], bf16, kind="ExternalInput")
    wdT_d = nc.dram_tensor("wdT", [EPC, 128, IC, H], bf16, kind="ExternalInput")
    out_d = nc.dram_tensor("out", [T // 4, H // 2], bf16, kind="ExternalOutput")

    with tile.TileContext(nc) as tc:
        _kernel_body(tc, xT_d, lg_d, wgT_d, wuT_d, wdT_d, out_d)
    nc.compile()
    return nc


def _kernel_body(tc, xT_d, lg_d, wgT_d, wuT_d, wdT_d, out_d):
    nc = tc.nc
    from contextlib import ExitStack

    ctx = ExitStack()
    with ctx:
        const = ctx.enter_context(tc.tile_pool(name="const", bufs=1))
        small = ctx.enter_context(tc.tile_pool(name="small", bufs=2))
        wg_pool = ctx.enter_context(tc.tile_pool(name="wg", bufs=4))
        wu_pool = ctx.enter_context(tc.tile_pool(name="wu", bufs=4))
        wd_pool = ctx.enter_context(tc.tile_pool(name="wd", bufs=3))
        act_pool = ctx.enter_context(tc.tile_pool(name="act", bufs=2))
        psg = ctx.enter_context(tc.tile_pool(name="psg", bufs=1, space="PSUM"))
        psu = ctx.enter_context(tc.tile_pool(name="psu", bufs=1, space="PSUM"))
        psd = ctx.enter_context(tc.tile_pool(name="psd", bufs=1, space="PSUM"))
        dram = ctx.enter_context(tc.tile_pool(name="dram", bufs=1, space="DRAM"))

        # ---- x (transposed, bf16) ----
        xt = const.tile([128, HC, T], bf16)  # x.T as [h_par, h_chunk, tok]
        nc.sync.dma_start(xt[:], xT_d.ap())

        # ---- routing: top-4 + softmax over selected logits ----
        lg = const.tile([T, E], f32)
        nc.scalar.dma_start(lg[:], lg_d.ap())

        work = small.tile([T, E], f32)
        nc.vector.tensor_copy(work[:], lg[:])
        negm0 = const.tile([T, 1], f32)
        mlast = const.tile([T, 1], f32)
        for k in range(TOPK):
            m = small.tile([T, 1], f32, tag="mk")
            nc.vector.tensor_reduce(m[:], work[:], axis=mybir.AxisListType.X, op=Alu.max)
            if k == 0:
                nc.vector.tensor_scalar_mul(negm0[:], m[:], -1.0)
            if k == TOPK - 1:
                nc.vector.tensor_copy(mlast[:], m[:])
            else:
                eq = small.tile([T, E], f32, tag="eq")
                nc.vector.tensor_scalar(eq[:], work[:], m[:], None, op0=Alu.is_equal)
                nc.vector.tensor_scalar(eq[:], eq[:], 1e30, None, op0=Alu.mult)
                nc.vector.tensor_tensor(work[:], work[:], eq[:], op=Alu.subtract)

        sel = small.tile([T, E], f32)
        nc.vector.tensor_scalar(sel[:], lg[:], mlast[:], None, op0=Alu.is_ge)
        ex = small.tile([T, E], f32)
        nc.scalar.activation(ex[:], lg[:], func=Act.Exp, bias=negm0[:], scale=1.0)
        nc.vector.tensor_tensor(ex[:], ex[:], sel[:], op=Alu.mult)
        den = small.tile([T, 1], f32)
        nc.vector.reduce_sum(den[:], ex[:], axis=mybir.AxisListType.X)
        rec = small.tile([T, 1], f32)
        nc.vector.reciprocal(rec[:], den[:])
        G = const.tile([T, E], f32)  # routing weights, local experts = cols 0..EPC-1
        nc.vector.tensor_scalar(G[:], ex[:], rec[:], None, op0=Alu.mult)

        # ---- G transposed + broadcast: gb_all[p, 64e + t] = G[t, e] for all p ----
        # DVE 32x32 block transpose: gt[e, j] = G[j, e], gt[32+e, j] = G[32+j, e]
        gt = const.tile([T, E], f32)
        nc.vector.transpose(gt[:], G[:])
        # gather the EPC expert rows (both token halves) onto partition 0
        grow = const.tile([1, EPC, 2, 32], f32)
        nc.scalar.dma_start(grow[:, :, 0, :], gt[0:EPC, 0:32])
        nc.scalar.dma_start(grow[:, :, 1, :], gt[32 : 32 + EPC, 0:32])
        gb_all = const.tile([128, EPC * T], f32)
        nc.gpsimd.partition_broadcast(gb_all[:], grow[:])
        gbs = [gb_all[:, T * e : T * (e + 1)] for e in range(EPC)]

        # ---- warm up the CC engine so the real ReduceScatter pays less
        # per-op setup; runs early, hidden under the weight streaming ----
        warm_sb = const.tile([8, 16], bf16)
        nc.vector.tensor_copy(warm_sb[:], lg[0:8, 0:16])
        warm_in = dram.tile([8, 16], bf16)
        nc.gpsimd.dma_start(warm_in[:], warm_sb[:])
        warm_out = dram.tile([8, 16], bf16)
        nc.gpsimd.collective_compute(
            "AllToAll",
            Alu.bypass,
            replica_groups=[list(range(NCORES))],
            ins=[warm_in.opt()],
            outs=[warm_out.opt()],
        )

        # ---- main expert loop ----
        cc_sb = const.tile([128, 1024], bf16)
        down_ps = psd.tile([128, 1024], f32)

        for e in range(EPC):
            last = e == EPC - 1
            gateT_ps = psg.tile([128, IC * T], f32, tag="g")
            upT_ps = psu.tile([128, IC * T], f32, tag="u")
            siluT = act_pool.tile([128, IC * T], f32, tag="silu")
            upG = act_pool.tile([128, IC * T], f32, tag="upG")
            mixT = act_pool.tile([128, IC * T], bf16, tag="mixT")

            # down-projection weights (own queue)
            wdts = []
            for (c0, cn) in WD_CH:
                wdt = wd_pool.tile([128, 4, H], bf16, tag="wd")
                nc.gpsimd.dma_start(wdt[:, :cn, :], wdT_d.ap()[e, :, c0 : c0 + cn, :])
                wdts.append(wdt)

            # gate/up weights, h-major streaming
            groups = GU_CH0 if e == 0 else GU_CH
            for (h0, hn) in groups:
                wgt = wg_pool.tile([128, 4, I], bf16, tag="wg")
                nc.sync.dma_start(wgt[:, :hn, :], wgT_d.ap()[e, :, h0 : h0 + hn, :])
                wut = wu_pool.tile([128, 4, I], bf16, tag="wu")
                nc.scalar.dma_start(wut[:, :hn, :], wuT_d.ap()[e, :, h0 : h0 + hn, :])
                for a in range(hn):
                    hc = h0 + a
                    xmv = xt[:, hc, :]
                    for it in range(IC):
                        first = hc == 0 and it in (0, 8)
                        nc.tensor.matmul(
                            gateT_ps[:, T * it : T * (it + 1)],
                            wgt[:, a, 128 * it : 128 * (it + 1)],
                            xmv,
                            start=first,
                            stop=(hc == HC - 1),
                        )
                        nc.tensor.matmul(
                            upT_ps[:, T * it : T * (it + 1)],
                            wut[:, a, 128 * it : 128 * (it + 1)],
                            xmv,
                            start=first,
                            stop=(hc == HC - 1),
                        )

            # mixed = silu(gate) * up * G[:, e]; down accumulates over experts
            for it in range(IC):
                sl = slice(T * it, T * (it + 1))
                nc.scalar.activation(siluT[:, sl], gateT_ps[:, sl], func=Act.Silu)
                nc.vector.tensor_tensor(upG[:, sl], upT_ps[:, sl], gbs[e], op=Alu.mult)
                nc.vector.tensor_tensor(mixT[:, sl], siluT[:, sl], upG[:, sl], op=Alu.mult)
                wdt = wdts[it // 4]
                icw = it % 4
                for th in range(2):
                    for b in range(2):
                        c0 = 1024 * th + 512 * b
                        nc.tensor.matmul(
                            down_ps[64 * th : 64 * th + 64, 512 * b : 512 * (b + 1)],
                            mixT[:, sl],
                            wdt[:, icw, c0 : c0 + 512],
                            start=(e == 0 and it == 0),
                            stop=(last and it == IC - 1),
                            tile_position=(0, 64 * th),
                            skip_group_check=True,
                        )

        # ---- tail: bf16 copy -> DRAM -> ReduceScatter over 8 cores ----
        cc_in = dram.tile([128, 1024], bf16)
        for hb in range(2):
            sl = slice(512 * hb, 512 * (hb + 1))
            nc.vector.tensor_copy(cc_sb[:, sl], down_ps[:, sl])
            nc.gpsimd.dma_start(cc_in[:, sl], cc_sb[:, sl])
        cc_out = dram.tile([NCORES, 16, 1024], bf16)
        nc.gpsimd.collective_compute(
            "AllToAll",
            Alu.bypass,
            replica_groups=[list(range(NCORES))],
            ins=[cc_in.opt()],
            outs=[cc_out.opt()],
        )
        # slot j = rank j's strip for MY tokens; sum all 8 locally
        strips = const.tile([16, NCORES, 1024], bf16)
        nc.gpsimd.dma_start(strips[:], cc_out[:].transpose([1, 0, 2]))
        racc = const.tile([16, 1024], f32)
        nc.vector.tensor_tensor(
            racc[:], strips[:, 0, :], strips[:, 1, :], op=Alu.add
        )
        for j in range(2, NCORES):
            nc.vector.tensor_tensor(racc[:], racc[:], strips[:, j, :], op=Alu.add)
        outw = const.tile([16, 1024], bf16)
        nc.vector.tensor_copy(outw[:], racc[:])
        nc.gpsimd.dma_start(out_d.ap(), outw[:])


_PROGRAM = None


def _get_program():
    global _PROGRAM
    if _PROGRAM is None:
        _PROGRAM = _build_program()
    return _PROGRAM


def _sbuf_layout(w, free):
    """[n, free_out, contract] expert weights -> [n, 128, chunks, free] bf16:
    transposed so the contraction dim is on partitions, tiled so each
    partition's data per chunk-group is one long contiguous DRAM run."""
    n, fo, contract = w.shape
    chunks = contract // 128
    a = w.transpose(0, 2, 1).reshape(n, chunks, 128, fo).transpose(0, 2, 1, 3)
    return np.ascontiguousarray(a.astype(_BF16))


def _make_in_maps(x, router_logits, w_gate, w_up, w_down):
    xT = np.ascontiguousarray(
        np.asarray(x, np.float32).T.reshape(HC, 128, T).transpose(1, 0, 2).astype(_BF16)
    )
    in_maps = []
    for c in range(NCORES):
        lo, hi = c * EPC, (c + 1) * EPC
        perm = list(range(lo, hi)) + [i for i in range(E) if not (lo <= i < hi)]
        lg_c = np.ascontiguousarray(router_logits[:, perm].astype(np.float32, copy=False))
        in_maps.append(
            {
                "xT": xT,
                "logits": lg_c,
                "wgT": _sbuf_layout(w_gate[lo:hi], I),
                "wuT": _sbuf_layout(w_up[lo:hi], I),
                "wdT": _sbuf_layout(w_down[lo:hi], H),
            }
        )
    return in_maps


def kernel(x, router_logits, w_gate, w_up, w_down, _trace=False, _results_out=None):
    x = np.asarray(x, dtype=np.float32)
    router_logits = np.asarray(router_logits, dtype=np.float32)
    w_gate = np.asarray(w_gate, dtype=np.float32)
    w_up = np.asarray(w_up, dtype=np.float32)
    w_down = np.asarray(w_down, dtype=np.float32)

    nc = _get_program()
    in_maps = _make_in_maps(x, router_logits, w_gate, w_up, w_down)
    res = bass_utils.run_bass_kernel_spmd(
        nc, in_maps, core_ids=list(range(NCORES)), trace=_trace
    )
    if _results_out is not None:
        _results_out.append(res)
    out = np.zeros((T, H), dtype=np.float32)
    for c in range(NCORES):
        shard = np.asarray(res.results[c]["out"]).astype(np.float32)  # [16, 1024]
        t0 = 16 * (c % 4)
        h0 = 1024 * (c // 4)
        out[t0 : t0 + 16, h0 : h0 + 1024] = shard
    return out[:, None, :].astype(np.float32)
